# revision 31
# baseline (speedup 1.0000x reference)
"""DeepSet (phi -> segment_sum -> rho) Bass kernel for 8 trn2 NeuronCores.

Sharding (per hint): data-parallel over segments. 16384 segments -> 8 cores x
2048 (segment-aligned row ranges via host searchsorted on sorted segment_ids).

v2: fp16 front end. The v1 profile showed ~530us/814us in the L2 + pool
matmuls: both use a per-tile fp32 stationary operand, which pays the 4-byte
double-pass LDWEIGHTS (no FWL: fp32 disqualifies fast-weight-load) plus
4 cycles/row streaming. Casting the front end to fp16 turns on FWL for the
128-column stationaries (h1a, onehot) and streams at 1 cycle/row: measured
rel err 1.0e-4 (CPU emulation) vs the 2e-3 harness budget.

Per-core dataflow (host-marshalled; T 128-row tiles per 128-seg window):
  - L1: z1[128, 512] = w1a[8, 128].T @ x[8, 512], fp16 operands (w1a padded
    to 128 cols for FWL; col 64 is the constant-one feature providing L2's
    bias via the contraction).
  - relu1 on ACT with per-partition bias -> h1a[65, 512] fp16.
  - L2: h2[128rows, 64] = h1a[65, 128].T @ w2a[65, 64] fp16 per tile - rows
    land on partitions, exactly what pooling needs.
  - relu2 on DVE (max with 0) -> h2t[128, 256] fp16 per 4-tile group.
  - onehot[128rows, 128segs] fp16 per tile = (idr == iota), one batched DVE
    is_equal per group (fp16 in/out: 2x DVE rate); padded rows have idr = -1.
  - pool: pooled[128segs, 64] += onehot[128, 128].T @ h2t[:, 64] fp16,
    PSUM-accumulated over the window's T tiles.
  - per window: PE-transpose pooled -> [64, 128segs] into a [65, 512]
    4-window chunk; row 64 = per-segment counts (host bincount, DMA'd).
  - tail per 512-seg chunk, all f32r (1 cyc/row at free dim 512): phi-L3
    (+ counts*b3 via the cnt row) then rho; p3's PSUM->SBUF move is a DVE
    copy, not an ACT Copy, so ACT never reloads its Relu table.
  - x is DMA'd per chunk (one [8, 4*PW] fp16 transfer) instead of per window.
Host gathers 8x[4, 2048] -> [16384, 4].
"""

import sys

import numpy as np

sys.path.insert(0, "/opt/trn_rl_repo")

import concourse.bass as bass  # noqa: E402
import concourse.mybir as mybir  # noqa: E402
import concourse.tile as tile  # noqa: E402
from concourse import bacc  # noqa: E402
from concourse.bass_utils import run_bass_kernel_spmd  # noqa: E402
from concourse.masks import make_identity  # noqa: E402

F32 = mybir.dt.float32
F32R = mybir.dt.float32r
F16 = mybir.dt.float16
I32 = mybir.dt.int32
AF = mybir.ActivationFunctionType

NUM_SEGMENTS = 16384
N_CORES = 8
SEG_PER_CORE = NUM_SEGMENTS // N_CORES  # 2048
WIN_SEGS = 128
N_WIN = SEG_PER_CORE // WIN_SEGS  # 16
STATE_DIM = 8
SDP = 32  # state dim zero-padded to the PE 32-row tile (fp16 K alignment)
HID = 64
H1P = 128  # h1 features (64 + ones row) zero-padded to 128 (fp16 K alignment)
OUT_DIM = 4
GRP = 4  # tiles per op-batch group (512 rows)
CHUNK = 512  # segs per batched rho-tail chunk (4 windows)
WPC = CHUNK // WIN_SEGS  # windows per chunk (4)

import os as _os

_BUILD_CACHE: dict[tuple, object] = {}


def _build_program(T: int, reps: int = 1, ablate=None):
    ab = set((ablate if ablate is not None else _os.environ.get("ABLATE", "")).split(","))
    oh_eng = _os.environ.get("ONEHOT_ENG", "dve")  # dve | pool
    r2_eng = _os.environ.get("RELU2_ENG", "dve")  # dve | act
    wbufs = int(_os.environ.get("WBUFS", "4"))
    xbufs = int(_os.environ.get("XBUFS", "2"))
    pool16 = _os.environ.get("POOL16", "1") == "1"  # fp16 pool operands
    l216 = _os.environ.get("L216", "1") == "1"  # fp16 L2 operands (h1a)
    padps = _os.environ.get("PADPS", "0") == "1"  # pad pooled/h2 PSUM to a full bank
    key = (T, reps, ",".join(sorted(ab)), oh_eng, r2_eng, wbufs, xbufs,
           pool16, l216, padps, _os.environ.get("DEBUG_POOLT", ""),
           _os.environ.get("LDWFIX", "1"))
    PDT = F16 if pool16 else F32
    LDT = F16 if l216 else F32
    if key in _BUILD_CACHE:
        return _BUILD_CACHE[key]
    assert T % GRP == 0
    PW = T * 128
    NG = T // GRP

    nc = bacc.Bacc("TRN2", target_bir_lowering=False, debug=False, num_devices=N_CORES)

    xT_d = nc.declare_dram_parameter("xT", [SDP, N_WIN * PW], F16, isOutput=False)
    idr_d = nc.declare_dram_parameter("idr", [128, N_WIN * T], F16, isOutput=False)
    cnt_d = nc.declare_dram_parameter("cnt", [1, SEG_PER_CORE], F32R, isOutput=False)
    w1a_d = nc.declare_dram_parameter("w1a", [SDP, 128], F16, isOutput=False)
    w2a_d = nc.declare_dram_parameter("w2a", [H1P, HID], LDT, isOutput=False)
    w3a_d = nc.declare_dram_parameter("w3a", [HID + 1, HID], F32R, isOutput=False)
    rw1_d = nc.declare_dram_parameter("rw1", [HID, HID], F32R, isOutput=False)
    rw2_d = nc.declare_dram_parameter("rw2", [HID, HID], F32R, isOutput=False)
    rw3_d = nc.declare_dram_parameter("rw3", [HID, OUT_DIM], F32R, isOutput=False)
    pb1a_d = nc.declare_dram_parameter("pb1a", [H1P, 1], F32, isOutput=False)
    rb1_d = nc.declare_dram_parameter("rb1", [HID, 1], F32, isOutput=False)
    rb2_d = nc.declare_dram_parameter("rb2", [HID, 1], F32, isOutput=False)
    rb3_d = nc.declare_dram_parameter("rb3", [OUT_DIM, 1], F32, isOutput=False)
    out_d = nc.declare_dram_parameter("out", [OUT_DIM, SEG_PER_CORE], F32, isOutput=True)
    dbg = _os.environ.get("DEBUG_POOLT", "") == "1"
    dbg_d = (
        nc.declare_dram_parameter("dbgT", [HID + 1, SEG_PER_CORE], F32, isOutput=True)
        if dbg
        else None
    )

    with tile.TileContext(nc) as tc:
        with (
            tc.tile_pool(name="const", bufs=1) as cpool,
            tc.tile_pool(name="xchunk", bufs=xbufs) as xpool,
            tc.tile_pool(name="work", bufs=wbufs) as wpool,
            tc.tile_pool(name="chunk", bufs=2) as chpool,
            tc.tile_pool(name="z1ps", bufs=2, space="PSUM") as z1ps,
            tc.tile_pool(name="h2ps", bufs=2, space="PSUM") as h2ps,
            tc.tile_pool(name="poolps", bufs=2, space="PSUM") as poolps,
            tc.tile_pool(name="tailps", bufs=2, space="PSUM") as tailps,
        ):
            def cload(name, shape, dram, dt=F32):
                t = cpool.tile(shape, dt, tag=name)
                nc.sync.dma_start(out=t[:], in_=dram[:])
                return t

            w1a = cload("w1a", [SDP, 128], w1a_d, F16)
            w2a = cload("w2a", [H1P, HID], w2a_d, LDT)
            w3a = cload("w3a", [HID + 1, HID], w3a_d, F32R)
            rw1 = cload("rw1", [HID, HID], rw1_d, F32R)
            rw2 = cload("rw2", [HID, HID], rw2_d, F32R)
            rw3 = cload("rw3", [HID, OUT_DIM], rw3_d, F32R)
            pb1a = cload("pb1a", [H1P, 1], pb1a_d)
            rb1 = cload("rb1", [HID, 1], rb1_d)
            rb2 = cload("rb2", [HID, 1], rb2_d)
            rb3 = cload("rb3", [OUT_DIM, 1], rb3_d)
            idr = cload("idr", [128, N_WIN * T], idr_d, F16)

            ident = cpool.tile([128, 128], F32, tag="ident")
            make_identity(nc, ident[:])
            iota_i = cpool.tile([128, GRP * 128], I32, tag="iota_i")
            nc.gpsimd.iota(
                iota_i[:], pattern=[[0, GRP], [1, 128]], base=0, channel_multiplier=0
            )
            iota4 = cpool.tile([128, GRP * 128], F16, tag="iota4")
            nc.vector.tensor_copy(out=iota4[:], in_=iota_i[:])

            oh = nc.gpsimd if oh_eng == "pool" else nc.vector

            for _rep in range(reps):
             for ch in range(SEG_PER_CORE // CHUNK):
                poolT = chpool.tile([HID + 1, CHUNK], F32R, tag="poolT")
                nc.sync.dma_start(
                    out=poolT[HID : HID + 1, :],
                    in_=cnt_d[:, ch * CHUNK : (ch + 1) * CHUNK],
                )
                xc = xpool.tile([SDP, WPC * PW], F16, tag="xc")
                if "xdma" not in ab:
                    nc.sync.dma_start(
                        out=xc[:], in_=xT_d[:, ch * WPC * PW : (ch + 1) * WPC * PW]
                    )
                for wl in range(WPC):
                    w = ch * WPC + wl

                    if "pool" in ab:
                        pooled_ps = None
                    else:
                        pooled_full = poolps.tile(
                            [WIN_SEGS, 512 if padps else HID], F32, tag="pool"
                        )
                        pooled_ps = pooled_full[:, :HID]

                    for g in range(NG):
                        gcols = slice(wl * PW + g * GRP * 128, wl * PW + (g + 1) * GRP * 128)
                        z1_ps = (None if "l1" in ab
                                 else z1ps.tile([128, GRP * 128], F32, tag="z1"))
                        if "l1" not in ab:
                            nc.tensor.matmul(
                                out=z1_ps[:], lhsT=w1a[:], rhs=xc[:, gcols],
                                start=True, stop=True,
                            )
                        if "relu1" not in ab:
                            # full 128 partitions: rows 65-127 are relu(0)+0 = 0,
                            # zero-padding the L2 contraction to the PE tile.
                            h1a = wpool.tile([H1P, GRP * 128], LDT, tag="h1a")
                            nc.scalar.activation(
                                out=h1a[:], in_=z1_ps[:], func=AF.Relu,
                                bias=pb1a[:],
                            )
                        else:
                            h1a = None

                        h2_ps = (None if "l2" in ab
                                 else h2ps.tile([128, GRP * HID], F32, tag="h2"))
                        for t in range(GRP) if "l2" not in ab else []:
                            nc.tensor.matmul(
                                out=h2_ps[:, t * HID : (t + 1) * HID],
                                lhsT=h1a[:, t * 128 : (t + 1) * 128],
                                rhs=w2a[:],
                                start=True,
                                stop=True,
                            )
                        h2t = None
                        if "relu2" not in ab:
                            h2t = wpool.tile([128, GRP * HID], PDT, tag="h2t")
                            if r2_eng == "act":
                                nc.scalar.activation(
                                    out=h2t[:], in_=h2_ps[:], func=AF.Relu, bias=0.0
                                )
                            else:
                                nc.vector.tensor_scalar(
                                    out=h2t[:], in0=h2_ps[:], scalar1=0.0, scalar2=None,
                                    op0=mybir.AluOpType.max,
                                )

                        onehot = None
                        c0 = w * T + g * GRP
                        if "onehot" not in ab:
                            onehot = wpool.tile([128, GRP * 128], PDT, tag="onehot")
                            oh.tensor_tensor(
                                out=onehot[:].rearrange("p (a b) -> p a b", b=128),
                                in0=idr[:, c0 : c0 + GRP].to_broadcast([128, GRP, 128]),
                                in1=iota4[:].rearrange("p (a b) -> p a b", b=128),
                                op=mybir.AluOpType.is_equal,
                            )
                        for t in range(GRP) if "pool" not in ab else []:
                            nc.tensor.matmul(
                                out=pooled_ps,
                                lhsT=onehot[:, t * 128 : (t + 1) * 128],
                                rhs=h2t[:, t * HID : (t + 1) * HID],
                                start=(g == 0 and t == 0),
                                stop=(g == NG - 1 and t == GRP - 1),
                            )

                    if "pool" not in ab:
                        pooled_sb = wpool.tile([WIN_SEGS, HID], F32, tag="pooled")
                        nc.vector.tensor_copy(out=pooled_sb[:], in_=pooled_ps)
                        poolT_ps = tailps.tile([HID, WIN_SEGS], F32, tag="tail")
                        nc.tensor.transpose(
                            out=poolT_ps[:], in_=pooled_sb[:], identity=ident[:]
                        )
                        nc.vector.tensor_copy(
                            out=poolT[:HID, wl * WIN_SEGS : (wl + 1) * WIN_SEGS],
                            in_=poolT_ps[:],
                        )

                if dbg:
                    dbg_sb = chpool.tile([HID + 1, CHUNK], F32, tag="dbgc")
                    nc.vector.tensor_copy(out=dbg_sb[:], in_=poolT[:])
                    nc.sync.dma_start(
                        out=dbg_d[:, ch * CHUNK : (ch + 1) * CHUNK], in_=dbg_sb[:]
                    )
                # batched phi-L3 + rho tail over this 512-seg chunk (all f32r)
                p3_ps = tailps.tile([HID, CHUNK], F32, tag="tail")
                nc.tensor.matmul(
                    out=p3_ps[:], lhsT=w3a[:], rhs=poolT[:], start=True, stop=True
                )
                p3 = chpool.tile([HID, CHUNK], F32R, tag="p3")
                nc.vector.tensor_copy(out=p3[:], in_=p3_ps[:])

                r1_ps = tailps.tile([HID, CHUNK], F32, tag="tail")
                nc.tensor.matmul(
                    out=r1_ps[:], lhsT=rw1[:], rhs=p3[:], start=True, stop=True
                )
                r1 = chpool.tile([HID, CHUNK], F32R, tag="r1")
                nc.scalar.activation(out=r1[:], in_=r1_ps[:], func=AF.Relu, bias=rb1[:])

                r2_ps = tailps.tile([HID, CHUNK], F32, tag="tail")
                nc.tensor.matmul(
                    out=r2_ps[:], lhsT=rw2[:], rhs=r1[:], start=True, stop=True
                )
                r2 = chpool.tile([HID, CHUNK], F32R, tag="r2")
                nc.scalar.activation(out=r2[:], in_=r2_ps[:], func=AF.Relu, bias=rb2[:])

                r3_ps = tailps.tile([OUT_DIM, CHUNK], F32, tag="tail")
                nc.tensor.matmul(
                    out=r3_ps[:], lhsT=rw3[:], rhs=r2[:], start=True, stop=True
                )
                out_sb = chpool.tile([OUT_DIM, CHUNK], F32, tag="outc")
                nc.vector.tensor_scalar(
                    out=out_sb[:], in0=r3_ps[:], scalar1=rb3[:], scalar2=None,
                    op0=mybir.AluOpType.add,
                )
                nc.sync.dma_start(
                    out=out_d[:, ch * CHUNK : (ch + 1) * CHUNK], in_=out_sb[:]
                )

    nc.compile()
    if _os.environ.get("LDWFIX", "1") == "1":
        _fix_ldw_waits(nc)
    _BUILD_CACHE[key] = nc
    return nc


def _fix_ldw_waits(nc):
    """Give every PE InstLdweights an explicit semaphore wait.

    The tile scheduler splits fp16 matmuls into Ldweights+Matmult and the
    sync passes elide waits that an earlier PE instruction already covers.
    That is only sound for strict in-order execution: the PE pulls Ldweights
    ahead of stalled instructions, so a wait-less Ldweights can stream a ring
    buffer before its producer has written it (observed as scrambled pooled
    sums). Re-waiting on a monotonic semaphore is free, so: each wait-less
    Ldweights gets a copy of the wait carried by the previous Ldweights of
    the same stationary tensor, or failing that the wait of its own Matmult
    (the instruction the wait was originally attached to, two slots later on
    the same queue - same blocking set, so no new deadlock is possible).
    """
    import re as _re

    f = nc.m.functions[0]
    pe = []
    for blk in f.blocks:
        for ins in blk.instructions:
            if str(getattr(ins, "engine", "")) == "EngineType.PE":
                pe.append(ins)

    def tname(ins):
        try:
            s = str(ins.ins[0])
        except Exception:
            return None
        m = _re.search(r"name='([^']+)'", s)
        return m.group(1) if m else None

    def waits_of(ins):
        si = ins.sync_info
        if si is None:
            return []
        return list(si.on_wait)

    # index of next InstMatmult for each position
    next_mm = [None] * len(pe)
    nm = None
    for i in range(len(pe) - 1, -1, -1):
        next_mm[i] = nm
        if type(pe[i]).__name__ == "InstMatmult":
            nm = pe[i]

    last_by_tensor = {}
    fixed = 0
    for i, ins in enumerate(pe):
        if type(ins).__name__ != "InstLdweights":
            continue
        t = tname(ins)
        w = waits_of(ins)
        if w:
            if t is not None:
                last_by_tensor[t] = w
            continue
        src = last_by_tensor.get(t)
        if not src and next_mm[i] is not None:
            src = waits_of(next_mm[i])
        if src:
            si = ins.sync_info
            upd = list(si.on_update) if si is not None else []
            ins.sync_info = mybir.SyncInfo(on_wait=[src[0]], on_update=upd)
            fixed += 1
    if _os.environ.get("LDWFIX_DEBUG"):
        print(f"_fix_ldw_waits: added waits to {fixed} Ldweights")


def _prep_inputs(neighbors: np.ndarray, segment_ids: np.ndarray):
    """Shard rows by 128-segment windows; pad each window to T 128-row tiles."""
    x = np.asarray(neighbors, dtype=np.float32)
    ids = np.asarray(segment_ids, dtype=np.int64)
    n_gwin = NUM_SEGMENTS // WIN_SEGS
    edges = np.searchsorted(ids, np.arange(0, NUM_SEGMENTS + 1, WIN_SEGS))
    wcnt = np.diff(edges)
    T = max(GRP, GRP * int(np.ceil(wcnt.max() / (128 * GRP))))
    PW = T * 128

    xT = np.zeros((N_CORES, SDP, N_WIN * PW), dtype=np.float16)
    idr = np.full((N_CORES, 128, N_WIN * T), -1.0, dtype=np.float16)
    counts = np.bincount(ids, minlength=NUM_SEGMENTS).astype(np.float32)
    cnt = counts.reshape(N_CORES, 1, SEG_PER_CORE)
    for g in range(n_gwin):
        c, wl = divmod(g, N_WIN)
        r0, r1 = int(edges[g]), int(edges[g + 1])
        n = r1 - r0
        if n == 0:
            continue
        base = wl * PW
        xT[c, :STATE_DIM, base : base + n] = x[r0:r1].T.astype(np.float16)
        rel = np.full(PW, -1.0, dtype=np.float32)
        rel[:n] = (ids[r0:r1] - g * WIN_SEGS).astype(np.float32)
        idr[c, :, wl * T : (wl + 1) * T] = rel.reshape(T, 128).T.astype(np.float16)
    return xT, idr, cnt, T


def prep_maps(inputs: dict):
    """Host-side marshalling: returns (T, in_maps per core)."""
    xT, idr, cnt, T = _prep_inputs(inputs["neighbors"], inputs["segment_ids"])
    f = lambda a: np.ascontiguousarray(np.asarray(a, dtype=np.float32))
    h = lambda a: np.ascontiguousarray(np.asarray(a, dtype=np.float16))
    col = lambda a: f(a).reshape(-1, 1)
    w1a = np.zeros((SDP, 128), np.float16)
    w1a[:STATE_DIM, :HID] = h(inputs["phi_W1"])
    pb1a = np.zeros((H1P, 1), np.float32)
    pb1a[:HID] = col(inputs["phi_b1"])
    pb1a[HID, 0] = 1.0
    w2dt = np.float16 if _os.environ.get("L216", "1") == "1" else np.float32
    w2a = np.zeros((H1P, HID), w2dt)
    w2a[:HID] = np.asarray(inputs["phi_W2"], dtype=w2dt)
    w2a[HID] = np.asarray(inputs["phi_b2"], dtype=w2dt)
    w3a = np.vstack([f(inputs["phi_W3"]), f(inputs["phi_b3"]).reshape(1, -1)])
    shared = {
        "w1a": w1a,
        "w2a": w2a,
        "w3a": w3a,
        "rw1": f(inputs["rho_W1"]),
        "rw2": f(inputs["rho_W2"]),
        "rw3": f(inputs["rho_W3"]),
        "pb1a": pb1a,
        "rb1": col(inputs["rho_b1"]),
        "rb2": col(inputs["rho_b2"]),
        "rb3": col(inputs["rho_b3"]),
    }
    in_maps = [
        {"xT": xT[c], "idr": idr[c], "cnt": cnt[c], **shared} for c in range(N_CORES)
    ]
    return T, in_maps


def kernel(**inputs):
    T, in_maps = prep_maps(inputs)
    nc = _build_program(T)
    res = run_bass_kernel_spmd(nc, in_maps, core_ids=list(range(N_CORES)))
    out = np.concatenate(
        [res.results[c]["out"].T for c in range(N_CORES)], axis=0
    ).astype(np.float32)
    return out


# revision 48
# speedup vs baseline: 1.3529x; 1.3529x over previous
"""DeepSet (phi -> segment_sum -> rho) Bass kernel for 8 trn2 NeuronCores.

Sharding (per hint): data-parallel over segments. 16384 segments -> 8 cores x
2048 (segment-aligned row ranges via host searchsorted on sorted segment_ids).

v3: fp16 front end. The v1 profile showed ~530us/814us in the L2 + pool
matmuls: both use a per-tile fp32 stationary operand, which pays the 4-byte
double-pass LDWEIGHTS (no FWL: fp32 disqualifies fast-weight-load) plus
4 cycles/row streaming. Casting the front end to fp16 turns on FWL for the
128-column stationaries (h1a, onehot) and streams at 1 cycle/row.

fp16 K-alignment (the v2->v3 fix): on hardware, fp16 matmuls execute the
tile_size-rounded contraction (K rounded up to 32/64/128), streaming stale
weight rows x out-of-AP SBUF for the padding rows - deterministic garbage
that CoreSim (exact-AP interpreter) does not reproduce. Every fp16
contraction is therefore zero-padded to its tile boundary: x/w1a rows 8-31
are host-zeroed (K=8 -> 32), and h1a/w2a rows 65-127 are zeroed on device /
host (K=65 -> 128), making the stray products exactly 0. Measured HW rel
err 3.0e-4 vs the 2e-3 harness budget.

Per-core dataflow (host-marshalled; T 128-row tiles per 128-seg window):
  - L1: z1[128, 512] = w1a[32, 128].T @ x[32, 512], fp16 (w1a zero-padded
    both ways; col 64 is the constant-one feature providing L2's bias via
    the contraction).
  - relu1 on ACT with per-partition bias -> h1a[128, 512] fp16 (rows 65-127
    = relu(0)+0 = 0: the L2 K-padding).
  - L2: h2[128rows, 64] = h1a[128, 128].T @ w2a[128, 64] fp16 per tile -
    rows land on partitions, exactly what pooling needs.
  - relu2 on DVE (max with 0) -> h2t[128, 256] fp16 per 4-tile group.
  - onehot[128rows, 128segs] fp16 = (idr == iota) on DVE (fp16 in/out: 2x
    DVE rate); padded rows have idr = -1 and match nothing.
  - pool: pooled[128segs, 64] += onehot[128, 128].T @ h2t[:, 64] fp16,
    PSUM-accumulated over the window's T tiles (both pool operands K=128,
    already aligned).
  - per window: PE-transpose pooled -> [64, 128segs] into a [65, 512]
    4-window chunk; row 64 = per-segment counts (host bincount, DMA'd).
  - tail per 512-seg chunk, all f32r (1 cyc/row at free dim 512): phi-L3
    (+ counts*b3 via the cnt row) then rho; p3's PSUM->SBUF move is a DVE
    copy, not an ACT Copy, so ACT never reloads its Relu table.
  - x is DMA'd per chunk (one [32, 4*PW] fp16 transfer) instead of per
    window.
  - post-compile, _fix_ldw_waits gives every PE Ldweights an explicit
    semaphore wait (the scheduler's wait elision assumes strict in-order
    execution, but the PE pulls Ldweights ahead of stalled matmuls).
Host gathers 8x[4, 2048] -> [16384, 4].
"""

import sys

import numpy as np

sys.path.insert(0, "/opt/trn_rl_repo")

import concourse.bass as bass  # noqa: E402
import concourse.mybir as mybir  # noqa: E402
import concourse.tile as tile  # noqa: E402
from concourse import bacc  # noqa: E402
from concourse.bass_utils import run_bass_kernel_spmd  # noqa: E402
from concourse.masks import make_identity  # noqa: E402

F32 = mybir.dt.float32
F32R = mybir.dt.float32r
F16 = mybir.dt.float16
I32 = mybir.dt.int32
AF = mybir.ActivationFunctionType

NUM_SEGMENTS = 16384
N_CORES = 8
SEG_PER_CORE = NUM_SEGMENTS // N_CORES  # 2048
WIN_SEGS = 128
N_WIN = SEG_PER_CORE // WIN_SEGS  # 16
STATE_DIM = 8
SDP = 32  # state dim zero-padded to the PE 32-row tile (fp16 K alignment)
HID = 64
H1P = 128  # h1 features (64 + ones row) zero-padded to 128 (fp16 K alignment)
OUT_DIM = 4
import os as _os_grp

GRP = int(_os_grp.environ.get("GRP", "4"))  # tiles per op-batch group
SUB = max(1, GRP // 4)  # 512-col sub-batches per group (PSUM bank limit)
CHUNK = 512  # segs per batched rho-tail chunk (4 windows)
WPC = CHUNK // WIN_SEGS  # windows per chunk (4)

import os as _os

_BUILD_CACHE: dict[tuple, object] = {}


def _build_program(T: int, reps: int = 1, ablate=None):
    ab = set((ablate if ablate is not None else _os.environ.get("ABLATE", "")).split(","))
    oh_eng = _os.environ.get("ONEHOT_ENG", "dve")  # dve | pool
    r2_eng = _os.environ.get("RELU2_ENG", "dve")  # dve | act
    wbufs = int(_os.environ.get("WBUFS", "4"))
    xbufs = int(_os.environ.get("XBUFS", "2"))
    pool16 = _os.environ.get("POOL16", "1") == "1"  # fp16 pool operands
    l216 = _os.environ.get("L216", "1") == "1"  # fp16 L2 operands (h1a)
    r1_split = _os.environ.get("RELU1_SPLIT", "0") == "1"  # relu1 on ACT+DVE halves
    r1_act2 = _os.environ.get("RELU1_ACT2", "0") == "1"  # relu1 as 2 ACT halves
    r2_split = _os.environ.get("RELU2_SPLIT", "0") == "1"  # relu2 as 2 DVE halves
    oh_win = _os.environ.get("OH_WIN", "1") == "1"  # one onehot op per window
    padps = _os.environ.get("PADPS", "0") == "1"  # pad pooled/h2 PSUM to a full bank
    key = (T, reps, ",".join(sorted(ab)), oh_eng, r2_eng, wbufs, xbufs,
           pool16, l216, padps, r1_split, r1_act2, r2_split, oh_win,
           _os.environ.get("DEBUG_POOLT", ""), _os.environ.get("LDWFIX", "1"))
    PDT = F16 if pool16 else F32
    LDT = F16 if l216 else F32
    if key in _BUILD_CACHE:
        return _BUILD_CACHE[key]
    assert T % GRP == 0
    PW = T * 128
    NG = T // GRP

    nc = bacc.Bacc("TRN2", target_bir_lowering=False, debug=False, num_devices=N_CORES)

    xT_d = nc.declare_dram_parameter("xT", [SDP, N_WIN * PW], F16, isOutput=False)
    idr_d = nc.declare_dram_parameter("idr", [128, N_WIN * T], F16, isOutput=False)
    cnt_d = nc.declare_dram_parameter("cnt", [1, SEG_PER_CORE], F32R, isOutput=False)
    w1a_d = nc.declare_dram_parameter("w1a", [SDP, 128], F16, isOutput=False)
    w2a_d = nc.declare_dram_parameter("w2a", [H1P, HID], LDT, isOutput=False)
    w3a_d = nc.declare_dram_parameter("w3a", [HID + 1, HID], F32R, isOutput=False)
    rw1_d = nc.declare_dram_parameter("rw1", [HID, HID], F32R, isOutput=False)
    rw2_d = nc.declare_dram_parameter("rw2", [HID, HID], F32R, isOutput=False)
    rw3_d = nc.declare_dram_parameter("rw3", [HID, OUT_DIM], F32R, isOutput=False)
    pb1a_d = nc.declare_dram_parameter("pb1a", [H1P, 1], F32, isOutput=False)
    rb1_d = nc.declare_dram_parameter("rb1", [HID, 1], F32, isOutput=False)
    rb2_d = nc.declare_dram_parameter("rb2", [HID, 1], F32, isOutput=False)
    rb3_d = nc.declare_dram_parameter("rb3", [OUT_DIM, 1], F32, isOutput=False)
    out_d = nc.declare_dram_parameter("out", [OUT_DIM, SEG_PER_CORE], F32, isOutput=True)
    dbg = _os.environ.get("DEBUG_POOLT", "") == "1"
    dbg_d = (
        nc.declare_dram_parameter("dbgT", [HID + 1, SEG_PER_CORE], F32, isOutput=True)
        if dbg
        else None
    )

    with tile.TileContext(nc) as tc:
        with (
            tc.tile_pool(name="const", bufs=1) as cpool,
            tc.tile_pool(name="xchunk", bufs=xbufs) as xpool,
            tc.tile_pool(name="work", bufs=wbufs) as wpool,
            tc.tile_pool(name="chunk", bufs=2) as chpool,
            tc.tile_pool(name="z1ps", bufs=2, space="PSUM") as z1ps,
            tc.tile_pool(name="h2ps", bufs=2, space="PSUM") as h2ps,
            tc.tile_pool(name="poolps", bufs=2, space="PSUM") as poolps,
            tc.tile_pool(name="tailps", bufs=2, space="PSUM") as tailps,
        ):
            def cload(name, shape, dram, dt=F32):
                t = cpool.tile(shape, dt, tag=name)
                nc.sync.dma_start(out=t[:], in_=dram[:])
                return t

            w1a = cload("w1a", [SDP, 128], w1a_d, F16)
            w2a = cload("w2a", [H1P, HID], w2a_d, LDT)
            w3a = cload("w3a", [HID + 1, HID], w3a_d, F32R)
            rw1 = cload("rw1", [HID, HID], rw1_d, F32R)
            rw2 = cload("rw2", [HID, HID], rw2_d, F32R)
            rw3 = cload("rw3", [HID, OUT_DIM], rw3_d, F32R)
            pb1a = cload("pb1a", [H1P, 1], pb1a_d)
            rb1 = cload("rb1", [HID, 1], rb1_d)
            rb2 = cload("rb2", [HID, 1], rb2_d)
            rb3 = cload("rb3", [OUT_DIM, 1], rb3_d)
            idr = cload("idr", [128, N_WIN * T], idr_d, F16)

            ident = cpool.tile([128, 128], F32, tag="ident")
            make_identity(nc, ident[:])
            iota_i = cpool.tile([128, GRP * 128], I32, tag="iota_i")
            nc.gpsimd.iota(
                iota_i[:], pattern=[[0, GRP], [1, 128]], base=0, channel_multiplier=0
            )
            iota4 = cpool.tile([128, GRP * 128], F16, tag="iota4")
            nc.vector.tensor_copy(out=iota4[:], in_=iota_i[:])
            if oh_win:
                iotaw_i = cpool.tile([128, T * 128], I32, tag="iotaw_i")
                nc.gpsimd.iota(
                    iotaw_i[:], pattern=[[0, T], [1, 128]], base=0,
                    channel_multiplier=0,
                )
                iotaw = cpool.tile([128, T * 128], F16, tag="iotaw")
                nc.vector.tensor_copy(out=iotaw[:], in_=iotaw_i[:])

            oh = nc.gpsimd if oh_eng == "pool" else nc.vector

            for _rep in range(reps):
             for ch in range(SEG_PER_CORE // CHUNK):
                poolT = chpool.tile([HID + 1, CHUNK], F32R, tag="poolT")
                nc.sync.dma_start(
                    out=poolT[HID : HID + 1, :],
                    in_=cnt_d[:, ch * CHUNK : (ch + 1) * CHUNK],
                )
                xc = xpool.tile([SDP, WPC * PW], F16, tag="xc")
                if "xdma" not in ab:
                    nc.sync.dma_start(
                        out=xc[:], in_=xT_d[:, ch * WPC * PW : (ch + 1) * WPC * PW]
                    )
                for wl in range(WPC):
                    w = ch * WPC + wl

                    if "pool" in ab:
                        pooled_ps = None
                    else:
                        pooled_full = poolps.tile(
                            [WIN_SEGS, 512 if padps else HID], F32, tag="pool"
                        )
                        pooled_ps = pooled_full[:, :HID]

                    onehot_w = None
                    if oh_win and "onehot" not in ab:
                        onehot_w = wpool.tile([128, T * 128], PDT, tag="ohw")
                        oh.tensor_tensor(
                            out=onehot_w[:].rearrange("p (a b) -> p a b", b=128),
                            in0=idr[:, w * T : (w + 1) * T].to_broadcast(
                                [128, T, 128]
                            ),
                            in1=iotaw[:].rearrange("p (a b) -> p a b", b=128),
                            op=mybir.AluOpType.is_equal,
                        )

                    for g in range(NG):
                        h1a = (None if "relu1" in ab
                               else wpool.tile([H1P, GRP * 128], LDT, tag="h1a"))
                        for s in range(SUB):
                            sc0 = s * (GRP // SUB) * 128
                            sc1 = (s + 1) * (GRP // SUB) * 128
                            gcols = slice(
                                wl * PW + g * GRP * 128 + sc0,
                                wl * PW + g * GRP * 128 + sc1,
                            )
                            z1_ps = (None if "l1" in ab
                                     else z1ps.tile([128, sc1 - sc0], F32, tag="z1"))
                            if "l1" not in ab:
                                nc.tensor.matmul(
                                    out=z1_ps[:], lhsT=w1a[:], rhs=xc[:, gcols],
                                    start=True, stop=True,
                                )
                            if "relu1" in ab:
                                continue
                            # full 128 partitions: rows 65-127 are relu(0)+0 = 0,
                            # zero-padding the L2 contraction to the PE tile.
                            hsl = slice(sc0, sc1)
                            if r1_split:
                                hcol = sc0 + (sc1 - sc0) // 2
                                nc.scalar.activation(
                                    out=h1a[:, sc0:hcol], in_=z1_ps[:, : hcol - sc0],
                                    func=AF.Relu, bias=pb1a[:],
                                )
                                nc.vector.tensor_scalar(
                                    out=h1a[:, hcol:sc1], in0=z1_ps[:, hcol - sc0 :],
                                    scalar1=pb1a[:], scalar2=0.0,
                                    op0=mybir.AluOpType.add,
                                    op1=mybir.AluOpType.max,
                                )
                            elif r1_act2:
                                hcol = sc0 + (sc1 - sc0) // 2
                                for hs, zs in (
                                    (slice(sc0, hcol), slice(0, hcol - sc0)),
                                    (slice(hcol, sc1), slice(hcol - sc0, sc1 - sc0)),
                                ):
                                    nc.scalar.activation(
                                        out=h1a[:, hs], in_=z1_ps[:, zs],
                                        func=AF.Relu, bias=pb1a[:],
                                    )
                            else:
                                nc.scalar.activation(
                                    out=h1a[:, hsl], in_=z1_ps[:], func=AF.Relu,
                                    bias=pb1a[:],
                                )

                        h2_ps = (None if "l2" in ab
                                 else h2ps.tile([128, GRP * HID], F32, tag="h2"))
                        for t in range(GRP) if "l2" not in ab else []:
                            nc.tensor.matmul(
                                out=h2_ps[:, t * HID : (t + 1) * HID],
                                lhsT=h1a[:, t * 128 : (t + 1) * 128],
                                rhs=w2a[:],
                                start=True,
                                stop=True,
                            )
                        h2t = None
                        if "relu2" not in ab:
                            h2t = wpool.tile([128, GRP * HID], PDT, tag="h2t")
                            if r2_eng == "act":
                                nc.scalar.activation(
                                    out=h2t[:], in_=h2_ps[:], func=AF.Relu, bias=0.0
                                )
                            elif r2_split:
                                hcol = GRP * HID // 2
                                for hs in (slice(0, hcol), slice(hcol, GRP * HID)):
                                    nc.vector.tensor_scalar(
                                        out=h2t[:, hs], in0=h2_ps[:, hs],
                                        scalar1=0.0, scalar2=None,
                                        op0=mybir.AluOpType.max,
                                    )
                            else:
                                nc.vector.tensor_scalar(
                                    out=h2t[:], in0=h2_ps[:], scalar1=0.0, scalar2=None,
                                    op0=mybir.AluOpType.max,
                                )

                        onehot = None
                        c0 = w * T + g * GRP
                        if not oh_win and "onehot" not in ab:
                            onehot = wpool.tile([128, GRP * 128], PDT, tag="onehot")
                            oh.tensor_tensor(
                                out=onehot[:].rearrange("p (a b) -> p a b", b=128),
                                in0=idr[:, c0 : c0 + GRP].to_broadcast([128, GRP, 128]),
                                in1=iota4[:].rearrange("p (a b) -> p a b", b=128),
                                op=mybir.AluOpType.is_equal,
                            )
                        for t in range(GRP) if "pool" not in ab else []:
                            if oh_win:
                                k = (g * GRP + t) * 128
                                oh_ap = onehot_w[:, k : k + 128]
                            else:
                                oh_ap = onehot[:, t * 128 : (t + 1) * 128]
                            nc.tensor.matmul(
                                out=pooled_ps,
                                lhsT=oh_ap,
                                rhs=h2t[:, t * HID : (t + 1) * HID],
                                start=(g == 0 and t == 0),
                                stop=(g == NG - 1 and t == GRP - 1),
                            )

                    if "pool" not in ab:
                        pooled_sb = wpool.tile([WIN_SEGS, HID], F32, tag="pooled")
                        nc.vector.tensor_copy(out=pooled_sb[:], in_=pooled_ps)
                        poolT_ps = tailps.tile([HID, WIN_SEGS], F32, tag="tail")
                        nc.tensor.transpose(
                            out=poolT_ps[:], in_=pooled_sb[:], identity=ident[:]
                        )
                        nc.vector.tensor_copy(
                            out=poolT[:HID, wl * WIN_SEGS : (wl + 1) * WIN_SEGS],
                            in_=poolT_ps[:],
                        )

                if dbg:
                    dbg_sb = chpool.tile([HID + 1, CHUNK], F32, tag="dbgc")
                    nc.vector.tensor_copy(out=dbg_sb[:], in_=poolT[:])
                    nc.sync.dma_start(
                        out=dbg_d[:, ch * CHUNK : (ch + 1) * CHUNK], in_=dbg_sb[:]
                    )
                # batched phi-L3 + rho tail over this 512-seg chunk (all f32r)
                p3_ps = tailps.tile([HID, CHUNK], F32, tag="tail")
                nc.tensor.matmul(
                    out=p3_ps[:], lhsT=w3a[:], rhs=poolT[:], start=True, stop=True
                )
                p3 = chpool.tile([HID, CHUNK], F32R, tag="p3")
                nc.vector.tensor_copy(out=p3[:], in_=p3_ps[:])

                r1_ps = tailps.tile([HID, CHUNK], F32, tag="tail")
                nc.tensor.matmul(
                    out=r1_ps[:], lhsT=rw1[:], rhs=p3[:], start=True, stop=True
                )
                r1 = chpool.tile([HID, CHUNK], F32R, tag="r1")
                nc.scalar.activation(out=r1[:], in_=r1_ps[:], func=AF.Relu, bias=rb1[:])

                r2_ps = tailps.tile([HID, CHUNK], F32, tag="tail")
                nc.tensor.matmul(
                    out=r2_ps[:], lhsT=rw2[:], rhs=r1[:], start=True, stop=True
                )
                r2 = chpool.tile([HID, CHUNK], F32R, tag="r2")
                nc.scalar.activation(out=r2[:], in_=r2_ps[:], func=AF.Relu, bias=rb2[:])

                r3_ps = tailps.tile([OUT_DIM, CHUNK], F32, tag="tail")
                nc.tensor.matmul(
                    out=r3_ps[:], lhsT=rw3[:], rhs=r2[:], start=True, stop=True
                )
                out_sb = chpool.tile([OUT_DIM, CHUNK], F32, tag="outc")
                nc.vector.tensor_scalar(
                    out=out_sb[:], in0=r3_ps[:], scalar1=rb3[:], scalar2=None,
                    op0=mybir.AluOpType.add,
                )
                nc.sync.dma_start(
                    out=out_d[:, ch * CHUNK : (ch + 1) * CHUNK], in_=out_sb[:]
                )

    nc.compile()
    if _os.environ.get("LDWFIX", "1") == "1":
        _fix_ldw_waits(nc)
    _BUILD_CACHE[key] = nc
    return nc


def _fix_ldw_waits(nc):
    """Give every PE InstLdweights an explicit semaphore wait.

    The tile scheduler splits fp16 matmuls into Ldweights+Matmult and the
    sync passes elide waits that an earlier PE instruction already covers.
    That is only sound for strict in-order execution: the PE pulls Ldweights
    ahead of stalled instructions, so a wait-less Ldweights can stream a ring
    buffer before its producer has written it (observed as scrambled pooled
    sums). Re-waiting on a monotonic semaphore is free, so: each wait-less
    Ldweights gets a copy of the wait carried by the previous Ldweights of
    the same stationary tensor, or failing that the wait of its own Matmult
    (the instruction the wait was originally attached to, two slots later on
    the same queue - same blocking set, so no new deadlock is possible).
    """
    import re as _re

    f = nc.m.functions[0]
    pe = []
    for blk in f.blocks:
        for ins in blk.instructions:
            if str(getattr(ins, "engine", "")) == "EngineType.PE":
                pe.append(ins)

    def tname(ins):
        try:
            s = str(ins.ins[0])
        except Exception:
            return None
        m = _re.search(r"name='([^']+)'", s)
        return m.group(1) if m else None

    def waits_of(ins):
        si = ins.sync_info
        if si is None:
            return []
        return list(si.on_wait)

    # index of next InstMatmult for each position
    next_mm = [None] * len(pe)
    nm = None
    for i in range(len(pe) - 1, -1, -1):
        next_mm[i] = nm
        if type(pe[i]).__name__ == "InstMatmult":
            nm = pe[i]

    last_by_tensor = {}
    fixed = 0
    for i, ins in enumerate(pe):
        if type(ins).__name__ != "InstLdweights":
            continue
        t = tname(ins)
        w = waits_of(ins)
        if w:
            if t is not None:
                last_by_tensor[t] = w
            continue
        src = last_by_tensor.get(t)
        if not src and next_mm[i] is not None:
            src = waits_of(next_mm[i])
        if src:
            si = ins.sync_info
            upd = list(si.on_update) if si is not None else []
            ins.sync_info = mybir.SyncInfo(on_wait=[src[0]], on_update=upd)
            fixed += 1
    if _os.environ.get("LDWFIX_DEBUG"):
        print(f"_fix_ldw_waits: added waits to {fixed} Ldweights")


def _prep_inputs(neighbors: np.ndarray, segment_ids: np.ndarray):
    """Shard rows by 128-segment windows; pad each window to T 128-row tiles."""
    x = np.asarray(neighbors, dtype=np.float32)
    ids = np.asarray(segment_ids, dtype=np.int64)
    n_gwin = NUM_SEGMENTS // WIN_SEGS
    edges = np.searchsorted(ids, np.arange(0, NUM_SEGMENTS + 1, WIN_SEGS))
    wcnt = np.diff(edges)
    T = max(GRP, GRP * int(np.ceil(wcnt.max() / (128 * GRP))))
    PW = T * 128

    xT = np.zeros((N_CORES, SDP, N_WIN * PW), dtype=np.float16)
    idr = np.full((N_CORES, 128, N_WIN * T), -1.0, dtype=np.float16)
    counts = np.bincount(ids, minlength=NUM_SEGMENTS).astype(np.float32)
    cnt = counts.reshape(N_CORES, 1, SEG_PER_CORE)
    for g in range(n_gwin):
        c, wl = divmod(g, N_WIN)
        r0, r1 = int(edges[g]), int(edges[g + 1])
        n = r1 - r0
        if n == 0:
            continue
        base = wl * PW
        xT[c, :STATE_DIM, base : base + n] = x[r0:r1].T.astype(np.float16)
        rel = np.full(PW, -1.0, dtype=np.float32)
        rel[:n] = (ids[r0:r1] - g * WIN_SEGS).astype(np.float32)
        idr[c, :, wl * T : (wl + 1) * T] = rel.reshape(T, 128).T.astype(np.float16)
    return xT, idr, cnt, T


def prep_maps(inputs: dict):
    """Host-side marshalling: returns (T, in_maps per core)."""
    xT, idr, cnt, T = _prep_inputs(inputs["neighbors"], inputs["segment_ids"])
    f = lambda a: np.ascontiguousarray(np.asarray(a, dtype=np.float32))
    h = lambda a: np.ascontiguousarray(np.asarray(a, dtype=np.float16))
    col = lambda a: f(a).reshape(-1, 1)
    w1a = np.zeros((SDP, 128), np.float16)
    w1a[:STATE_DIM, :HID] = h(inputs["phi_W1"])
    pb1a = np.zeros((H1P, 1), np.float32)
    pb1a[:HID] = col(inputs["phi_b1"])
    pb1a[HID, 0] = 1.0
    w2dt = np.float16 if _os.environ.get("L216", "1") == "1" else np.float32
    w2a = np.zeros((H1P, HID), w2dt)
    w2a[:HID] = np.asarray(inputs["phi_W2"], dtype=w2dt)
    w2a[HID] = np.asarray(inputs["phi_b2"], dtype=w2dt)
    w3a = np.vstack([f(inputs["phi_W3"]), f(inputs["phi_b3"]).reshape(1, -1)])
    shared = {
        "w1a": w1a,
        "w2a": w2a,
        "w3a": w3a,
        "rw1": f(inputs["rho_W1"]),
        "rw2": f(inputs["rho_W2"]),
        "rw3": f(inputs["rho_W3"]),
        "pb1a": pb1a,
        "rb1": col(inputs["rho_b1"]),
        "rb2": col(inputs["rho_b2"]),
        "rb3": col(inputs["rho_b3"]),
    }
    in_maps = [
        {"xT": xT[c], "idr": idr[c], "cnt": cnt[c], **shared} for c in range(N_CORES)
    ]
    return T, in_maps


def kernel(**inputs):
    T, in_maps = prep_maps(inputs)
    nc = _build_program(T)
    res = run_bass_kernel_spmd(nc, in_maps, core_ids=list(range(N_CORES)))
    out = np.concatenate(
        [res.results[c]["out"].T for c in range(N_CORES)], axis=0
    ).astype(np.float32)
    return out


# revision 51
# speedup vs baseline: 1.8009x; 1.3312x over previous
"""DeepSet (phi -> segment_sum -> rho) Bass kernel for 8 trn2 NeuronCores.

Sharding (per hint): data-parallel over segments. 16384 segments -> 8 cores x
2048 (segment-aligned row ranges via host searchsorted on sorted segment_ids).

v3: fp16 front end. The v1 profile showed ~530us/814us in the L2 + pool
matmuls: both use a per-tile fp32 stationary operand, which pays the 4-byte
double-pass LDWEIGHTS (no FWL: fp32 disqualifies fast-weight-load) plus
4 cycles/row streaming. Casting the front end to fp16 turns on FWL for the
128-column stationaries (h1a, onehot) and streams at 1 cycle/row.

fp16 K-alignment (the v2->v3 fix): on hardware, fp16 matmuls execute the
tile_size-rounded contraction (K rounded up to 32/64/128), streaming stale
weight rows x out-of-AP SBUF for the padding rows - deterministic garbage
that CoreSim (exact-AP interpreter) does not reproduce. Every fp16
contraction is therefore zero-padded to its tile boundary: x/w1a rows 8-31
are host-zeroed (K=8 -> 32), and h1a/w2a rows 65-127 are zeroed on device /
host (K=65 -> 128), making the stray products exactly 0. Measured HW rel
err 3.0e-4 vs the 2e-3 harness budget.

Per-core dataflow (host-marshalled; T 128-row tiles per 128-seg window):
  - L1: z1[128, 512] = w1a[32, 128].T @ x[32, 512], fp16 (w1a zero-padded
    both ways; col 64 is the constant-one feature providing L2's bias via
    the contraction).
  - relu1 on ACT with per-partition bias -> h1a[128, 512] fp16 (rows 65-127
    = relu(0)+0 = 0: the L2 K-padding).
  - L2: h2[128rows, 64] = h1a[128, 128].T @ w2a[128, 64] fp16 per tile -
    rows land on partitions, exactly what pooling needs.
  - relu2 on DVE (max with 0) -> h2t[128, 256] fp16 per 4-tile group.
  - onehot[128rows, 128segs] fp16 = (idr == iota) on DVE (fp16 in/out: 2x
    DVE rate); padded rows have idr = -1 and match nothing.
  - pool: pooled[128segs, 64] += onehot[128, 128].T @ h2t[:, 64] fp16,
    PSUM-accumulated over the window's T tiles (both pool operands K=128,
    already aligned).
  - per window: PE-transpose pooled -> [64, 128segs] into a [65, 512]
    4-window chunk; row 64 = per-segment counts (host bincount, DMA'd).
  - tail per 512-seg chunk, all f32r (1 cyc/row at free dim 512): phi-L3
    (+ counts*b3 via the cnt row) then rho; p3's PSUM->SBUF move is a DVE
    copy, not an ACT Copy, so ACT never reloads its Relu table.
  - x is DMA'd per chunk (one [32, 4*PW] fp16 transfer) instead of per
    window.
  - post-compile, _fix_ldw_waits gives every PE Ldweights an explicit
    semaphore wait (the scheduler's wait elision assumes strict in-order
    execution, but the PE pulls Ldweights ahead of stalled matmuls).
Host gathers 8x[4, 2048] -> [16384, 4].
"""

import sys

import numpy as np

sys.path.insert(0, "/opt/trn_rl_repo")

import concourse.bass as bass  # noqa: E402
import concourse.mybir as mybir  # noqa: E402
import concourse.tile as tile  # noqa: E402
from concourse import bacc  # noqa: E402
from concourse.bass_utils import run_bass_kernel_spmd  # noqa: E402
from concourse.masks import make_identity  # noqa: E402

F32 = mybir.dt.float32
F32R = mybir.dt.float32r
F16 = mybir.dt.float16
I32 = mybir.dt.int32
AF = mybir.ActivationFunctionType

NUM_SEGMENTS = 16384
N_CORES = 8
SEG_PER_CORE = NUM_SEGMENTS // N_CORES  # 2048
WIN_SEGS = 128
N_WIN = SEG_PER_CORE // WIN_SEGS  # 16
STATE_DIM = 8
SDP = 32  # state dim zero-padded to the PE 32-row tile (fp16 K alignment)
HID = 64
H1P = 128  # h1 features (64 + ones row) zero-padded to 128 (fp16 K alignment)
OUT_DIM = 4
import os as _os_grp

GRP = int(_os_grp.environ.get("GRP", "4"))  # tiles per op-batch group
SUB = max(1, GRP // 4)  # 512-col sub-batches per group (PSUM bank limit)
CHUNK = 512  # segs per batched rho-tail chunk (4 windows)
WPC = CHUNK // WIN_SEGS  # windows per chunk (4)

import os as _os

_BUILD_CACHE: dict[tuple, object] = {}


def _build_program(T: int, reps: int = 1, ablate=None):
    ab = set((ablate if ablate is not None else _os.environ.get("ABLATE", "")).split(","))
    oh_eng = _os.environ.get("ONEHOT_ENG", "dve")  # dve | pool
    r2_eng = _os.environ.get("RELU2_ENG", "dve")  # dve | act
    wbufs = int(_os.environ.get("WBUFS", "4"))
    xbufs = int(_os.environ.get("XBUFS", "2"))
    pool16 = _os.environ.get("POOL16", "1") == "1"  # fp16 pool operands
    l216 = _os.environ.get("L216", "1") == "1"  # fp16 L2 operands (h1a)
    r1_split = _os.environ.get("RELU1_SPLIT", "0") == "1"  # relu1 on ACT+DVE halves
    r1_act2 = _os.environ.get("RELU1_ACT2", "0") == "1"  # relu1 as 2 ACT halves
    r2_split = _os.environ.get("RELU2_SPLIT", "0") == "1"  # relu2 as 2 DVE halves
    oh_win = _os.environ.get("OH_WIN", "1") == "1"  # one onehot op per window
    padps = _os.environ.get("PADPS", "0") == "1"  # pad pooled/h2 PSUM to a full bank
    key = (T, reps, ",".join(sorted(ab)), oh_eng, r2_eng, wbufs, xbufs,
           pool16, l216, padps, r1_split, r1_act2, r2_split, oh_win,
           _os.environ.get("DEBUG_POOLT", ""), _os.environ.get("LDWFIX", "1"))
    PDT = F16 if pool16 else F32
    LDT = F16 if l216 else F32
    if key in _BUILD_CACHE:
        return _BUILD_CACHE[key]
    assert T % GRP == 0
    PW = T * 128
    NG = T // GRP

    nc = bacc.Bacc("TRN2", target_bir_lowering=False, debug=False, num_devices=N_CORES)

    xT_d = nc.declare_dram_parameter("xT", [SDP, N_WIN * PW], F16, isOutput=False)
    idr_d = nc.declare_dram_parameter("idr", [128, N_WIN * T], F16, isOutput=False)
    cnt_d = nc.declare_dram_parameter("cnt", [1, SEG_PER_CORE], F32R, isOutput=False)
    w1a_d = nc.declare_dram_parameter("w1a", [SDP, 128], F16, isOutput=False)
    w2a_d = nc.declare_dram_parameter("w2a", [H1P, HID], LDT, isOutput=False)
    w3a_d = nc.declare_dram_parameter("w3a", [HID + 1, HID], F32R, isOutput=False)
    rw1_d = nc.declare_dram_parameter("rw1", [HID, HID], F32R, isOutput=False)
    rw2_d = nc.declare_dram_parameter("rw2", [HID, HID], F32R, isOutput=False)
    rw3_d = nc.declare_dram_parameter("rw3", [HID, OUT_DIM], F32R, isOutput=False)
    pb1a_d = nc.declare_dram_parameter("pb1a", [H1P, 1], F32, isOutput=False)
    rb1_d = nc.declare_dram_parameter("rb1", [HID, 1], F32, isOutput=False)
    rb2_d = nc.declare_dram_parameter("rb2", [HID, 1], F32, isOutput=False)
    rb3_d = nc.declare_dram_parameter("rb3", [OUT_DIM, 1], F32, isOutput=False)
    out_d = nc.declare_dram_parameter("out", [OUT_DIM, SEG_PER_CORE], F32, isOutput=True)
    dbg = _os.environ.get("DEBUG_POOLT", "") == "1"
    dbg_d = (
        nc.declare_dram_parameter("dbgT", [HID + 1, SEG_PER_CORE], F32, isOutput=True)
        if dbg
        else None
    )

    with tile.TileContext(nc) as tc:
        with (
            tc.tile_pool(name="const", bufs=1) as cpool,
            tc.tile_pool(name="xchunk", bufs=xbufs) as xpool,
            tc.tile_pool(name="work", bufs=wbufs) as wpool,
            tc.tile_pool(name="chunk", bufs=2) as chpool,
            tc.tile_pool(name="z1ps", bufs=2, space="PSUM") as z1ps,
            tc.tile_pool(name="h2ps", bufs=2, space="PSUM") as h2ps,
            tc.tile_pool(name="poolps", bufs=2, space="PSUM") as poolps,
            tc.tile_pool(name="tailps", bufs=2, space="PSUM") as tailps,
        ):
            def cload(name, shape, dram, dt=F32, eng=None):
                t = cpool.tile(shape, dt, tag=name)
                (eng or nc.sync).dma_start(out=t[:], in_=dram[:])
                return t

            # compute-critical loads first (the single-shot critical path pays
            # each queue's serial ~1us descriptor generation); tail weights
            # go on the otherwise-idle ACT/DVE hwdge queues.
            w1a = cload("w1a", [SDP, 128], w1a_d, F16)
            pb1a = cload("pb1a", [H1P, 1], pb1a_d)
            w2a = cload("w2a", [H1P, HID], w2a_d, LDT)
            idr = cload("idr", [128, N_WIN * T], idr_d, F16, eng=nc.scalar)
            w3a = cload("w3a", [HID + 1, HID], w3a_d, F32R, eng=nc.scalar)
            rw1 = cload("rw1", [HID, HID], rw1_d, F32R, eng=nc.scalar)
            rw2 = cload("rw2", [HID, HID], rw2_d, F32R, eng=nc.scalar)
            rw3 = cload("rw3", [HID, OUT_DIM], rw3_d, F32R, eng=nc.scalar)
            rb1 = cload("rb1", [HID, 1], rb1_d, eng=nc.scalar)
            rb2 = cload("rb2", [HID, 1], rb2_d, eng=nc.scalar)
            rb3 = cload("rb3", [OUT_DIM, 1], rb3_d, eng=nc.scalar)

            ident = cpool.tile([128, 128], F32, tag="ident")
            make_identity(nc, ident[:])
            iota_i = cpool.tile([128, GRP * 128], I32, tag="iota_i")
            nc.gpsimd.iota(
                iota_i[:], pattern=[[0, GRP], [1, 128]], base=0, channel_multiplier=0
            )
            iota4 = cpool.tile([128, GRP * 128], F16, tag="iota4")
            nc.vector.tensor_copy(out=iota4[:], in_=iota_i[:])
            if oh_win:
                iotaw_i = cpool.tile([128, T * 128], I32, tag="iotaw_i")
                nc.gpsimd.iota(
                    iotaw_i[:], pattern=[[0, T], [1, 128]], base=0,
                    channel_multiplier=0,
                )
                iotaw = cpool.tile([128, T * 128], F16, tag="iotaw")
                nc.vector.tensor_copy(out=iotaw[:], in_=iotaw_i[:])

            oh = nc.gpsimd if oh_eng == "pool" else nc.vector

            for _rep in range(reps):
             for ch in range(SEG_PER_CORE // CHUNK):
                poolT = chpool.tile([HID + 1, CHUNK], F32R, tag="poolT")
                nc.sync.dma_start(
                    out=poolT[HID : HID + 1, :],
                    in_=cnt_d[:, ch * CHUNK : (ch + 1) * CHUNK],
                )
                xc = xpool.tile([SDP, WPC * PW], F16, tag="xc")
                if "xdma" not in ab:
                    nc.sync.dma_start(
                        out=xc[:], in_=xT_d[:, ch * WPC * PW : (ch + 1) * WPC * PW]
                    )
                for wl in range(WPC):
                    w = ch * WPC + wl

                    if "pool" in ab:
                        pooled_ps = None
                    else:
                        pooled_full = poolps.tile(
                            [WIN_SEGS, 512 if padps else HID], F32, tag="pool"
                        )
                        pooled_ps = pooled_full[:, :HID]

                    onehot_w = None
                    if oh_win and "onehot" not in ab:
                        onehot_w = wpool.tile([128, T * 128], PDT, tag="ohw")
                        oh.tensor_tensor(
                            out=onehot_w[:].rearrange("p (a b) -> p a b", b=128),
                            in0=idr[:, w * T : (w + 1) * T].to_broadcast(
                                [128, T, 128]
                            ),
                            in1=iotaw[:].rearrange("p (a b) -> p a b", b=128),
                            op=mybir.AluOpType.is_equal,
                        )

                    for g in range(NG):
                        h1a = (None if "relu1" in ab
                               else wpool.tile([H1P, GRP * 128], LDT, tag="h1a"))
                        for s in range(SUB):
                            sc0 = s * (GRP // SUB) * 128
                            sc1 = (s + 1) * (GRP // SUB) * 128
                            gcols = slice(
                                wl * PW + g * GRP * 128 + sc0,
                                wl * PW + g * GRP * 128 + sc1,
                            )
                            z1_ps = (None if "l1" in ab
                                     else z1ps.tile([128, sc1 - sc0], F32, tag="z1"))
                            if "l1" not in ab:
                                nc.tensor.matmul(
                                    out=z1_ps[:], lhsT=w1a[:], rhs=xc[:, gcols],
                                    start=True, stop=True,
                                )
                            if "relu1" in ab:
                                continue
                            # full 128 partitions: rows 65-127 are relu(0)+0 = 0,
                            # zero-padding the L2 contraction to the PE tile.
                            hsl = slice(sc0, sc1)
                            if r1_split:
                                hcol = sc0 + (sc1 - sc0) // 2
                                nc.scalar.activation(
                                    out=h1a[:, sc0:hcol], in_=z1_ps[:, : hcol - sc0],
                                    func=AF.Relu, bias=pb1a[:],
                                )
                                nc.vector.tensor_scalar(
                                    out=h1a[:, hcol:sc1], in0=z1_ps[:, hcol - sc0 :],
                                    scalar1=pb1a[:], scalar2=0.0,
                                    op0=mybir.AluOpType.add,
                                    op1=mybir.AluOpType.max,
                                )
                            elif r1_act2:
                                hcol = sc0 + (sc1 - sc0) // 2
                                for hs, zs in (
                                    (slice(sc0, hcol), slice(0, hcol - sc0)),
                                    (slice(hcol, sc1), slice(hcol - sc0, sc1 - sc0)),
                                ):
                                    nc.scalar.activation(
                                        out=h1a[:, hs], in_=z1_ps[:, zs],
                                        func=AF.Relu, bias=pb1a[:],
                                    )
                            else:
                                nc.scalar.activation(
                                    out=h1a[:, hsl], in_=z1_ps[:], func=AF.Relu,
                                    bias=pb1a[:],
                                )

                        h2_ps = (None if "l2" in ab
                                 else h2ps.tile([128, GRP * HID], F32, tag="h2"))
                        for t in range(GRP) if "l2" not in ab else []:
                            nc.tensor.matmul(
                                out=h2_ps[:, t * HID : (t + 1) * HID],
                                lhsT=h1a[:, t * 128 : (t + 1) * 128],
                                rhs=w2a[:],
                                start=True,
                                stop=True,
                            )
                        h2t = None
                        if "relu2" not in ab:
                            h2t = wpool.tile([128, GRP * HID], PDT, tag="h2t")
                            if r2_eng == "act":
                                nc.scalar.activation(
                                    out=h2t[:], in_=h2_ps[:], func=AF.Relu, bias=0.0
                                )
                            elif r2_split:
                                hcol = GRP * HID // 2
                                for hs in (slice(0, hcol), slice(hcol, GRP * HID)):
                                    nc.vector.tensor_scalar(
                                        out=h2t[:, hs], in0=h2_ps[:, hs],
                                        scalar1=0.0, scalar2=None,
                                        op0=mybir.AluOpType.max,
                                    )
                            else:
                                nc.vector.tensor_scalar(
                                    out=h2t[:], in0=h2_ps[:], scalar1=0.0, scalar2=None,
                                    op0=mybir.AluOpType.max,
                                )

                        onehot = None
                        c0 = w * T + g * GRP
                        if not oh_win and "onehot" not in ab:
                            onehot = wpool.tile([128, GRP * 128], PDT, tag="onehot")
                            oh.tensor_tensor(
                                out=onehot[:].rearrange("p (a b) -> p a b", b=128),
                                in0=idr[:, c0 : c0 + GRP].to_broadcast([128, GRP, 128]),
                                in1=iota4[:].rearrange("p (a b) -> p a b", b=128),
                                op=mybir.AluOpType.is_equal,
                            )
                        for t in range(GRP) if "pool" not in ab else []:
                            if oh_win:
                                k = (g * GRP + t) * 128
                                oh_ap = onehot_w[:, k : k + 128]
                            else:
                                oh_ap = onehot[:, t * 128 : (t + 1) * 128]
                            nc.tensor.matmul(
                                out=pooled_ps,
                                lhsT=oh_ap,
                                rhs=h2t[:, t * HID : (t + 1) * HID],
                                start=(g == 0 and t == 0),
                                stop=(g == NG - 1 and t == GRP - 1),
                            )

                    if "pool" not in ab:
                        pooled_sb = wpool.tile([WIN_SEGS, HID], F32, tag="pooled")
                        nc.vector.tensor_copy(out=pooled_sb[:], in_=pooled_ps)
                        poolT_ps = tailps.tile([HID, WIN_SEGS], F32, tag="tail")
                        nc.tensor.transpose(
                            out=poolT_ps[:], in_=pooled_sb[:], identity=ident[:]
                        )
                        nc.vector.tensor_copy(
                            out=poolT[:HID, wl * WIN_SEGS : (wl + 1) * WIN_SEGS],
                            in_=poolT_ps[:],
                        )

                if dbg:
                    dbg_sb = chpool.tile([HID + 1, CHUNK], F32, tag="dbgc")
                    nc.vector.tensor_copy(out=dbg_sb[:], in_=poolT[:])
                    nc.sync.dma_start(
                        out=dbg_d[:, ch * CHUNK : (ch + 1) * CHUNK], in_=dbg_sb[:]
                    )
                # batched phi-L3 + rho tail over this 512-seg chunk (all f32r)
                p3_ps = tailps.tile([HID, CHUNK], F32, tag="tail")
                nc.tensor.matmul(
                    out=p3_ps[:], lhsT=w3a[:], rhs=poolT[:], start=True, stop=True
                )
                p3 = chpool.tile([HID, CHUNK], F32R, tag="p3")
                nc.vector.tensor_copy(out=p3[:], in_=p3_ps[:])

                r1_ps = tailps.tile([HID, CHUNK], F32, tag="tail")
                nc.tensor.matmul(
                    out=r1_ps[:], lhsT=rw1[:], rhs=p3[:], start=True, stop=True
                )
                r1 = chpool.tile([HID, CHUNK], F32R, tag="r1")
                nc.scalar.activation(out=r1[:], in_=r1_ps[:], func=AF.Relu, bias=rb1[:])

                r2_ps = tailps.tile([HID, CHUNK], F32, tag="tail")
                nc.tensor.matmul(
                    out=r2_ps[:], lhsT=rw2[:], rhs=r1[:], start=True, stop=True
                )
                r2 = chpool.tile([HID, CHUNK], F32R, tag="r2")
                nc.scalar.activation(out=r2[:], in_=r2_ps[:], func=AF.Relu, bias=rb2[:])

                r3_ps = tailps.tile([OUT_DIM, CHUNK], F32, tag="tail")
                nc.tensor.matmul(
                    out=r3_ps[:], lhsT=rw3[:], rhs=r2[:], start=True, stop=True
                )
                out_sb = chpool.tile([OUT_DIM, CHUNK], F32, tag="outc")
                nc.vector.tensor_scalar(
                    out=out_sb[:], in0=r3_ps[:], scalar1=rb3[:], scalar2=None,
                    op0=mybir.AluOpType.add,
                )
                nc.sync.dma_start(
                    out=out_d[:, ch * CHUNK : (ch + 1) * CHUNK], in_=out_sb[:]
                )

    nc.compile()
    if _os.environ.get("LDWFIX", "1") == "1":
        _fix_ldw_waits(nc)
    _BUILD_CACHE[key] = nc
    return nc


def _fix_ldw_waits(nc):
    """Give every PE InstLdweights an explicit semaphore wait.

    The tile scheduler splits fp16 matmuls into Ldweights+Matmult and the
    sync passes elide waits that an earlier PE instruction already covers.
    That is only sound for strict in-order execution: the PE pulls Ldweights
    ahead of stalled instructions, so a wait-less Ldweights can stream a ring
    buffer before its producer has written it (observed as scrambled pooled
    sums). Re-waiting on a monotonic semaphore is free, so: each wait-less
    Ldweights gets a copy of the wait carried by the previous Ldweights of
    the same stationary tensor, or failing that the wait of its own Matmult
    (the instruction the wait was originally attached to, two slots later on
    the same queue - same blocking set, so no new deadlock is possible).
    """
    import re as _re

    f = nc.m.functions[0]
    pe = []
    for blk in f.blocks:
        for ins in blk.instructions:
            if str(getattr(ins, "engine", "")) == "EngineType.PE":
                pe.append(ins)

    def tname(ins):
        try:
            s = str(ins.ins[0])
        except Exception:
            return None
        m = _re.search(r"name='([^']+)'", s)
        return m.group(1) if m else None

    def waits_of(ins):
        si = ins.sync_info
        if si is None:
            return []
        return list(si.on_wait)

    # index of next InstMatmult for each position
    next_mm = [None] * len(pe)
    nm = None
    for i in range(len(pe) - 1, -1, -1):
        next_mm[i] = nm
        if type(pe[i]).__name__ == "InstMatmult":
            nm = pe[i]

    last_by_tensor = {}
    fixed = 0
    for i, ins in enumerate(pe):
        if type(ins).__name__ != "InstLdweights":
            continue
        t = tname(ins)
        w = waits_of(ins)
        if w:
            if t is not None:
                last_by_tensor[t] = w
            continue
        src = last_by_tensor.get(t)
        if not src and next_mm[i] is not None:
            src = waits_of(next_mm[i])
        if src:
            si = ins.sync_info
            upd = list(si.on_update) if si is not None else []
            ins.sync_info = mybir.SyncInfo(on_wait=[src[0]], on_update=upd)
            fixed += 1
    if _os.environ.get("LDWFIX_DEBUG"):
        print(f"_fix_ldw_waits: added waits to {fixed} Ldweights")


def _prep_inputs(neighbors: np.ndarray, segment_ids: np.ndarray):
    """Shard rows by 128-segment windows; pad each window to T 128-row tiles."""
    x = np.asarray(neighbors, dtype=np.float32)
    ids = np.asarray(segment_ids, dtype=np.int64)
    n_gwin = NUM_SEGMENTS // WIN_SEGS
    edges = np.searchsorted(ids, np.arange(0, NUM_SEGMENTS + 1, WIN_SEGS))
    wcnt = np.diff(edges)
    T = max(GRP, GRP * int(np.ceil(wcnt.max() / (128 * GRP))))
    PW = T * 128

    xT = np.zeros((N_CORES, SDP, N_WIN * PW), dtype=np.float16)
    idr = np.full((N_CORES, 128, N_WIN * T), -1.0, dtype=np.float16)
    counts = np.bincount(ids, minlength=NUM_SEGMENTS).astype(np.float32)
    cnt = counts.reshape(N_CORES, 1, SEG_PER_CORE)
    for g in range(n_gwin):
        c, wl = divmod(g, N_WIN)
        r0, r1 = int(edges[g]), int(edges[g + 1])
        n = r1 - r0
        if n == 0:
            continue
        base = wl * PW
        xT[c, :STATE_DIM, base : base + n] = x[r0:r1].T.astype(np.float16)
        rel = np.full(PW, -1.0, dtype=np.float32)
        rel[:n] = (ids[r0:r1] - g * WIN_SEGS).astype(np.float32)
        idr[c, :, wl * T : (wl + 1) * T] = rel.reshape(T, 128).T.astype(np.float16)
    return xT, idr, cnt, T


def prep_maps(inputs: dict):
    """Host-side marshalling: returns (T, in_maps per core)."""
    xT, idr, cnt, T = _prep_inputs(inputs["neighbors"], inputs["segment_ids"])
    f = lambda a: np.ascontiguousarray(np.asarray(a, dtype=np.float32))
    h = lambda a: np.ascontiguousarray(np.asarray(a, dtype=np.float16))
    col = lambda a: f(a).reshape(-1, 1)
    w1a = np.zeros((SDP, 128), np.float16)
    w1a[:STATE_DIM, :HID] = h(inputs["phi_W1"])
    pb1a = np.zeros((H1P, 1), np.float32)
    pb1a[:HID] = col(inputs["phi_b1"])
    pb1a[HID, 0] = 1.0
    w2dt = np.float16 if _os.environ.get("L216", "1") == "1" else np.float32
    w2a = np.zeros((H1P, HID), w2dt)
    w2a[:HID] = np.asarray(inputs["phi_W2"], dtype=w2dt)
    w2a[HID] = np.asarray(inputs["phi_b2"], dtype=w2dt)
    w3a = np.vstack([f(inputs["phi_W3"]), f(inputs["phi_b3"]).reshape(1, -1)])
    shared = {
        "w1a": w1a,
        "w2a": w2a,
        "w3a": w3a,
        "rw1": f(inputs["rho_W1"]),
        "rw2": f(inputs["rho_W2"]),
        "rw3": f(inputs["rho_W3"]),
        "pb1a": pb1a,
        "rb1": col(inputs["rho_b1"]),
        "rb2": col(inputs["rho_b2"]),
        "rb3": col(inputs["rho_b3"]),
    }
    in_maps = [
        {"xT": xT[c], "idr": idr[c], "cnt": cnt[c], **shared} for c in range(N_CORES)
    ]
    return T, in_maps


def kernel(**inputs):
    T, in_maps = prep_maps(inputs)
    nc = _build_program(T)
    res = run_bass_kernel_spmd(nc, in_maps, core_ids=list(range(N_CORES)))
    out = np.concatenate(
        [res.results[c]["out"].T for c in range(N_CORES)], axis=0
    ).astype(np.float32)
    return out
